# revision 64
# baseline (speedup 1.0000x reference)
"""Trainium2 Bass kernel for CoreSageLayer (GNN mean-aggregate + 3-way linear).

Computation (reference):
    mask = (adj == 1)                      # [N, N] 0/1
    deg  = mask.sum(axis=1)                # [N]
    x1   = (mask @ x) / deg[:, None]       # [N, F]
    out[k] = concat([x1, x], 1) @ W[k] + bias   # [3, N, O]

Distribution: row-shard adj / x1 / out over nodes across 8 cores; replicate
x and weights; no collectives (rows independent).

Device schedule per core (nodes NB=1024, 8 node-tiles of 128):
  stage 1 (per node-tile j): PSUM[128, 256] accumulates 32 fp8 DoubleRow
      matmuls (256-deep contraction each):
      lhsT = adjT chunk [128 p, 2, 128 n] (host pre-transposed, fp8 exact),
      rhs  = x chunk [128 p, 2, 256 f] (fp8).
  finalize j: x1 = psum * rec_j (host-computed 1/deg, DVE, bf16 out),
      PE-transpose x1 (bf16) into x1T, then stage 2 in bf16:
      out[k, j] = [x1, x]^T-contracted matmuls, fp32 PSUM -> bf16 out DMA.
  A PE warm-up burst (matmuls on the identity) during the initial DMA fill
  ramps the tensor engine to full clock before real data lands.
"""

import os
import sys

sys.path.insert(0, "/opt/trn_rl_repo")

import numpy as np

N = 8192
F = 256
O = 256
NCORES = 8
NB = N // NCORES          # nodes per core (1024)
JT = NB // 128            # node tiles per core (8)
MCH2 = N // 256           # 256-deep contraction chunks (32)
NWARM = int(os.environ.get("KNWARM", "18"))   # PE warm-up matmuls
MASK_BUFS = int(os.environ.get("KMASKBUFS", "3"))
ACC_BUFS = int(os.environ.get("KACCBUFS", "2"))


def _patch_tile_drain():
    """This container's walrus allows only one sync-wait per CTRL instruction;
    split the Tile kernel-tail drain's waits onto single-wait no-fuse NoOps."""
    import concourse.tile as tile
    from concourse import mybir
    from concourse.tile import ScopedClock

    if getattr(tile.TileContext, "_drain_split_patched", False):
        return

    def _drain_and_barrier(self, tick_clock, wait_clock):
        nc = self.nc
        drain_inst = nc.sync.drain()
        wait_clock.add_sem_waits(
            drain_inst.ins, ScopedClock({None: tick_clock.global_clock})
        )
        si = drain_inst.ins.sync_info
        if si is not None and len(si.on_wait) > 1:
            waits = list(si.on_wait)
            drain_inst.ins.sync_info = mybir.SyncInfo(
                on_wait=[waits[0]], on_update=list(si.on_update)
            )
            for w in waits[1:]:
                nop = nc.sync.nop(nofuse=True, hint="split_wait")
                nop.ins.sync_info = mybir.SyncInfo(on_wait=[w], on_update=[])
        nc.all_engine_barrier()
        assert self.sems is not None
        popped = nc._tile_sem_poison_stack.pop()
        assert popped is self._sem_poison
        nc.clear_and_free_semaphores(list(self.sems.allocated().values()))
        nc.all_engine_barrier()

    tile.TileContext._drain_and_barrier = _drain_and_barrier
    tile.TileContext._drain_split_patched = True

    # Same walrus limitation, general case: any instruction that Tile gave
    # >1 sem-wait (e.g. a DMA with both RAW and WAR deps) fails codegen.
    # Split surplus waits onto fresh single-wait NoOps emitted just before
    # the instruction on the same engine, at the serialized-BIR level.
    import concourse.bass as bass
    import orjson

    _orig_to_json_bytes = bass.Bass.to_json_bytes

    def _to_json_bytes_split(self):
        m = orjson.loads(_orig_to_json_bytes(self))
        ctr = 0
        for fn in m.get("functions", []):
            for bb in fn.get("blocks", []):
                insts = bb.get("instructions", [])
                new = []
                for inst in insts:
                    si = inst.get("sync_info")
                    waits = (si or {}).get("on_wait") or []
                    if len(waits) > 1:
                        for w in waits[:-1]:
                            ctr += 1
                            new.append({
                                "name": f"SWNOP-{ctr}",
                                "opcode": "NoOp",
                                "engine": inst["engine"],
                                "ins": [],
                                "outs": [],
                                "sync_info": {"on_wait": [w], "on_update": []},
                            })
                        si["on_wait"] = [waits[-1]]
                    new.append(inst)
                bb["instructions"] = new
        return orjson.dumps(m)

    bass.Bass.to_json_bytes = _to_json_bytes_split


def build_bass(with_bias: bool):
    import concourse.bass as bass
    import concourse.tile as tile
    from concourse import mybir
    from concourse.masks import make_identity

    _patch_tile_drain()

    fp8 = mybir.dt.float8e4
    bf16 = mybir.dt.bfloat16
    f32 = mybir.dt.float32
    DR = mybir.MatmulPerfMode.DoubleRow

    nc = bass.Bass()
    maskt = nc.dram_tensor("maskt", [JT, 128, MCH2 * 2, 128], fp8,
                           kind="ExternalInput")
    xp = nc.dram_tensor("xp", [128, MCH2 * 2, F], fp8, kind="ExternalInput")
    # x^T for the stage-2 x path, fp8 value + residual + scaled copy:
    # r=0: fp8(x), r=1: fp8(x - fp8(x)), r=2: fp8(x/32). Three DoubleRow
    # matmuls (r0+r1 vs w2_hi, r2 vs 32*(w2-w2_hi)) reproduce x@W2 better
    # than bf16 at fp8 speed
    xt = nc.dram_tensor("xt", [128, 3, 2, NB], fp8, kind="ExternalInput")
    # stage-2 weights [p, k, i, o] halves (f = i*128 + p within the half):
    # w1 (x1 path), w2h/w2l (x path, value + residual)
    w1 = nc.dram_tensor("w1", [128, 3, 2, O], fp8, kind="ExternalInput")
    w2h = nc.dram_tensor("w2h", [128, 3, 2, O], fp8, kind="ExternalInput")
    w2l = nc.dram_tensor("w2l", [128, 3, 2, O], fp8, kind="ExternalInput")
    recs = nc.dram_tensor("recs", [128, JT], f32, kind="ExternalInput")
    if with_bias:
        biasr = nc.dram_tensor("biasr", [128, O], f32, kind="ExternalInput")
    out = nc.dram_tensor("out", [NB, 3, O], bf16, kind="ExternalOutput")

    with tile.TileContext(nc) as tc:
        with (
            tc.tile_pool(name="const", bufs=1) as const_pool,
            tc.tile_pool(name="mask", bufs=MASK_BUFS) as mask_pool,
            tc.tile_pool(name="work", bufs=3) as work_pool,
            tc.tile_pool(name="psum1", bufs=2, space="PSUM") as psum1_pool,
            tc.tile_pool(name="psumt", bufs=2, space="PSUM") as psumt_pool,
            tc.tile_pool(name="psum2", bufs=2, space="PSUM") as psum2_pool,
        ):
            # identity first: the PE warm-up below only depends on it
            # (bf16: the hw fp8 transpose needs strided outputs, so x1 is
            # transposed in bf16 and cast to fp8 by the PSUM->SBUF copy)
            identity = const_pool.tile([128, 128], bf16)
            make_identity(nc, identity)

            # ---- front DMAs. The cost model serializes transfers on the DMA
            # device, so split mask j=0 and x into pieces interleaved across
            # the SP and ACT HWDGE rings: stage-1 chunk c needs mask piece
            # c//8 and x piece c//4.
            mt0 = mask_pool.tile([128, MCH2 * 2, 128], fp8, tag="mt", name="mt0")
            mt1 = mask_pool.tile([128, MCH2 * 2, 128], fp8, tag="mt",
                                 name="mt1")
            xp_sb = const_pool.tile([128, MCH2 * 2, F], fp8)
            # j0/j1 masks + x interleaved across the three DMA streams so
            # stage-1 tiles 0 and 1 are both fed by ~6us
            mw = MCH2
            xw = MCH2 * 2 // 4
            nc.sync.dma_start(mt0[:, :mw], maskt[0][:, :mw])
            nc.sync.dma_start(mt0[:, mw:], maskt[0][:, mw:])
            nc.sync.dma_start(mt1[:, :mw], maskt[1][:, :mw])
            nc.scalar.dma_start(xp_sb[:, 0 * xw:1 * xw], xp[:, 0 * xw:1 * xw])
            nc.scalar.dma_start(xp_sb[:, 1 * xw:2 * xw], xp[:, 1 * xw:2 * xw])
            nc.scalar.dma_start(mt1[:, mw:], maskt[1][:, mw:])
            nc.gpsimd.dma_start(xp_sb[:, 2 * xw:3 * xw], xp[:, 2 * xw:3 * xw])
            nc.gpsimd.dma_start(xp_sb[:, 3 * xw:4 * xw], xp[:, 3 * xw:4 * xw])

            # PE warm-up: ~1.5us of dummy matmuls so the pstate ramp
            # completes while the front DMAs stream in. Reuses the stage-1
            # accumulator tag (PE executes in order, so the WAW is free)
            # to keep all 8 PSUM banks available for the pipeline.
            for i in range(NWARM):
                wm = psum2_pool.tile([128, 128], f32, tag="warm", bufs=1)
                nc.tensor.matmul(wm[:], identity[:], identity[:],
                                 start=True, stop=True)

            def stage1(j, mt, split=False):
                # split=True: accumulate the two f-halves as separate PSUM
                # groups so finalize can start on half 0 early (used for the
                # last node-tile, where the x1 chain is the kernel tail)
                ps = psum1_pool.tile([128, F], f32, tag="acc", bufs=ACC_BUFS)
                halves = [(0, F)] if not split else [(0, 128), (128, F)]
                for lo, hi in halves:
                    for c in range(MCH2):
                        nc.tensor.matmul(
                            ps[:, lo:hi],
                            mt[:, 2 * c:2 * c + 2],
                            xp_sb[:, 2 * c:2 * c + 2, lo:hi],
                            start=(c == 0),
                            stop=(c == MCH2 - 1),
                            perf_mode=DR,
                        )
                return ps

            ps0 = stage1(0, mt0)

            # stage-2 constants: emitted after stage1(0), used by finalize(0)
            recs_sb = const_pool.tile([128, JT], f32)
            nc.gpsimd.dma_start(recs_sb[:], recs[:])
            xt_sb = const_pool.tile([128, 3, 2, NB], fp8)
            nc.gpsimd.dma_start(xt_sb[:], xt[:])
            w1_sb = const_pool.tile([128, 3, 2, O], fp8)
            nc.gpsimd.dma_start(w1_sb[:], w1[:])
            w2h_sb = const_pool.tile([128, 3, 2, O], fp8)
            nc.gpsimd.dma_start(w2h_sb[:], w2h[:])
            w2l_sb = const_pool.tile([128, 3, 2, O], fp8)
            nc.gpsimd.dma_start(w2l_sb[:], w2l[:])
            if with_bias:
                bias_sb = const_pool.tile([128, O], f32)
                nc.gpsimd.dma_start(bias_sb[:], biasr[:])
            # x1T: [p, h, n] with row f = h*128+p; filled per j
            x1t_sb = const_pool.tile([128, 2, NB], fp8)

            def load_mask(j):
                mt = mask_pool.tile([128, MCH2 * 2, 128], fp8, tag="mt",
                                    name=f"mt{j}")
                eng = nc.sync if j % 2 == 0 else nc.scalar
                eng.dma_start(mt[:], maskt[j])
                return mt

            def finalize(j, ps):
                jcols = slice(j * 128, (j + 1) * 128)
                x1 = work_pool.tile([128, F], bf16, tag="x1")
                # both scales first: DVE then runs scale-h0, scale-h1,
                # copy-h0, copy-h1 with the PE transposes overlapped, instead
                # of stalling scale-h1 behind copy-h0 in its queue
                for h in range(2):
                    hs = slice(h * 128, (h + 1) * 128)
                    nc.vector.tensor_scalar_mul(x1[:, hs], ps[:, hs],
                                                recs_sb[:, j:j + 1])
                pts = []
                for h in range(2):
                    hs = slice(h * 128, (h + 1) * 128)
                    pt = psumt_pool.tile([128, 128], bf16, tag="pt",
                                         name=f"pt{j}_{h}")
                    nc.tensor.transpose(pt[:], x1[:, hs], identity[:])
                    pts.append(pt)
                for h in range(2):
                    nc.vector.tensor_copy(x1t_sb[:, h, jcols], pts[h][:])
                # stage-2: fp8 DoubleRow, 256-deep contraction per matmul
                ot = work_pool.tile([128, 3, O], bf16, tag="ot")
                jrows = slice(j * 128, (j + 1) * 128)
                last = j == JT - 1

                def mm_x1(k, po, start, stop):
                    nc.tensor.matmul(po[:], x1t_sb[:, :, jcols],
                                     w1_sb[:, k], start=start, stop=stop,
                                     perf_mode=DR)

                def mm_x(k, po, start, stop):
                    # x@W2 ~ (x_hi + x_lo)·w_hi + (x/32)·(32·w_lo),
                    # all fp8 DoubleRow (the scale keeps w_lo out of the
                    # e4m3 subnormal floor; measured better than bf16)
                    nc.tensor.matmul(po[:], xt_sb[:, 0, :, jcols],
                                     w2h_sb[:, k], start=start,
                                     stop=False, perf_mode=DR)
                    nc.tensor.matmul(po[:], xt_sb[:, 1, :, jcols],
                                     w2h_sb[:, k], start=False,
                                     stop=False, perf_mode=DR)
                    nc.tensor.matmul(po[:], xt_sb[:, 2, :, jcols],
                                     w2l_sb[:, k], start=False,
                                     stop=stop, perf_mode=DR)

                def emit_out(k, po):
                    if with_bias:
                        nc.vector.tensor_add(ot[:, k], po[:], bias_sb[:])
                    elif last and k == 2:
                        # the final copy: DVE, so it doesn't queue behind
                        # k0/k1 on the ACT FIFO in the kernel tail
                        nc.vector.tensor_copy(ot[:, k], po[:])
                    else:
                        # ACT engine is otherwise idle; keep DVE for x1/x1t
                        nc.scalar.copy(ot[:, k], po[:])

                out_eng = [nc.gpsimd, nc.scalar, nc.sync]
                for k in range(3):
                    po = psum2_pool.tile([128, O], f32, tag="po")
                    if last:
                        # x-path first: it doesn't depend on this tile's x1,
                        # so the PE isn't stalled on the x1 chain in the tail
                        mm_x(k, po, True, False)
                        mm_x1(k, po, False, True)
                    else:
                        mm_x1(k, po, True, False)
                        mm_x(k, po, False, True)
                    emit_out(k, po)
                    if last:
                        # split the final write per k across the three DMA
                        # streams so the tail drains in parallel
                        out_eng[k].dma_start(out[jrows, k], ot[:, k])
                if not last:
                    nc.gpsimd.dma_start(out[jrows], ot[:])

            # software-pipeline by one node-tile so PE never stalls on the
            # DVE x1 scaling between stage-1 accumulation and stage 2
            prev = (0, ps0)
            mt_next = mt1
            for j in range(1, JT):
                mt = mt_next
                mt_next = load_mask(j + 1) if j + 1 < JT else None
                ps = stage1(j, mt, split=(j == JT - 1))
                finalize(*prev)
                prev = (j, ps)
            finalize(*prev)

    return nc


_cached = {}


def _get_bass(with_bias: bool):
    if with_bias not in _cached:
        _cached[with_bias] = build_bass(with_bias)
    return _cached[with_bias]


def _host_prep(x, adj, weight, bias):
    import ml_dtypes

    fp8 = ml_dtypes.float8_e4m3
    bf16 = ml_dtypes.bfloat16
    x = np.asarray(x, dtype=np.float32)
    adj = np.asarray(adj)
    weight = np.asarray(weight, dtype=np.float32)
    bias = np.asarray(bias, dtype=np.float32)

    with_bias = bool(np.any(bias))

    # mask as fp8 bytes without a float cast: 1.0 is 0x38 in e4m3
    mask_u8 = (adj == 1).view(np.uint8) * np.uint8(0x38)      # [N, N]
    deg = (adj == 1).sum(axis=1, dtype=np.int32)              # [N]
    rec_full = (1.0 / deg.astype(np.float32))

    # replicated: x in stage-1 rhs layout [128 p][c 32][i 2][F], m=c*256+i*128+p
    x_fp8 = x.astype(fp8)
    xp_t = np.ascontiguousarray(
        x_fp8.reshape(MCH2, 2, 128, F).transpose(2, 0, 1, 3)
    ).reshape(128, MCH2 * 2, F)

    # stage-2 weight halves [p, k, i, o] with f = (g*256) + i*128 + p;
    # the x-path half is sent as fp8 value + fp8 residual
    wr = weight.reshape(3, 2, 2, 128, O).transpose(3, 0, 1, 2, 4)
    w1_t = np.ascontiguousarray(wr[:, :, 0]).astype(fp8)
    w2 = np.ascontiguousarray(wr[:, :, 1])
    w2h_t = w2.astype(fp8)
    w2l_t = ((w2 - w2h_t.astype(np.float32)) * 32.0).astype(fp8)
    bias_r = np.broadcast_to(bias, (128, O)).copy() if with_bias else None

    in_maps = []
    for c in range(NCORES):
        rows = slice(c * NB, (c + 1) * NB)
        # adjT shard [j][p][c][i][n]: element = mask[node j*128+n, m=c*256+i*128+p]
        a = np.ascontiguousarray(
            mask_u8[rows].T.reshape(MCH2, 2, 128, JT, 128)
            .transpose(3, 2, 0, 1, 4)
        ).reshape(JT, 128, MCH2 * 2, 128).view(fp8)
        xs = x[rows]
        xs_hi = xs.astype(fp8)
        xs_lo = (xs - xs_hi.astype(np.float32)).astype(fp8)
        xs_32 = (xs / 32.0).astype(fp8)
        xt_c = np.ascontiguousarray(
            np.stack([xs_hi, xs_lo, xs_32])     # [r, NB, F]
            .transpose(2, 0, 1)                 # [F, r, NB]
            .reshape(2, 128, 3, NB)             # [i, p, r, NB]
            .transpose(1, 2, 0, 3))             # [p, r, i, NB]
        rec_c = np.ascontiguousarray(
            rec_full[rows].reshape(JT, 128).T)
        m = {"maskt": a, "xp": xp_t, "xt": xt_c, "w1": w1_t, "w2h": w2h_t,
             "w2l": w2l_t, "recs": rec_c}
        if with_bias:
            m["biasr"] = bias_r
        in_maps.append(m)
    return in_maps, with_bias


def run(x, adj, weight, bias, trace=False, trace_kwargs=None):
    """Shard, run on 8 cores, gather. Returns (out_full, BassKernelResults)."""
    from concourse.bass_utils import run_bass_kernel_spmd

    in_maps, with_bias = _host_prep(x, adj, weight, bias)
    nc = _get_bass(with_bias)
    res = run_bass_kernel_spmd(
        nc, in_maps, list(range(NCORES)), trace=trace, **(trace_kwargs or {})
    )
    out_full = np.empty((3, N, O), dtype=np.float32)
    for c in range(NCORES):
        # device out is [NB, 3, O] bf16
        out_full[:, c * NB:(c + 1) * NB, :] = (
            res.results[c]["out"].astype(np.float32).transpose(1, 0, 2)
        )
    return out_full, res


def kernel(g, x, adj, weight, bias):
    out, _ = run(x, adj, weight, bias)
    return out
